# revision 1
# baseline (speedup 1.0000x reference)
"""Trainium2 Bass kernel for nn_Attention (GQA causal attention + RoPE), v2.

Full problem: x[4,2048,2048] -> attention(16 q heads / 8 kv heads, head_dim
128, llama RoPE, causal) -> out[4,2048,2048], fp32.

Sharding: core = batch*2 + head_group (tensor-parallel over heads x
data-parallel over batch). Host sums the two head-group partials per batch
(the Wo all-reduce).

Per-core kernel, mixed precision tuned to the TRN2 cost model:
- Q/K/V projections: 3-term fp8(e4m3) residual DoubleRow matmuls
  (x ~ xhi+xlo, W ~ whi+wlo; terms hi*hi + hi*lo + lo*hi), 0.75x the
  bf16 cost at ~0.2% error. x scaled by 32, W by 512 on host.
- RoPE: fp16 on DVE (ACT copy-with-scale evacuates PSUM at 1/16384).
- scores: fp16 matmuls into 2-bank PSUM swaths; the causal mask is ADDED
  in PSUM by tiny fp8 DoubleRow bias matmuls ((-15 I) @ (15 U) = -225 on
  anti-causal entries) so exp can run on full [128,1024] swaths.
- softmax: one wide exp per (head-pair, k-block) on ACT; denominators
  by DVE accumulation of e + gpsimd partition_all_reduce; fp16
  reciprocal gives the broadcast row directly; normalize fused with the
  PSUM->SBUF evac on DVE (scaled x16 into an fp8 hi/lo attn pair).
- scores are computed per HEAD-PAIR (both q heads sharing a kv head) in
  one matmul; output projection is 3-term fp8 residual DoubleRow.
"""

import math
from contextlib import ExitStack

import numpy as np
import ml_dtypes

import concourse.bass as bass
import concourse.bass_isa as bass_isa
import concourse.mybir as mybir
import concourse.tile as tile
from concourse import bacc
from concourse.bass_utils import run_bass_kernel_spmd

F32 = mybir.dt.float32
FP16 = mybir.dt.float16
FP8 = mybir.dt.float8e4
F8NP = ml_dtypes.float8_e4m3
DR = mybir.MatmulPerfMode.DoubleRow

B, S, D = 4, 2048, 2048
H, KVH, HD = 16, 8, 128
NG = 2
NQ = H // NG           # 8 q heads per core
NKV = KVH // NG        # 4 kv heads per core
REP = NQ // NKV
N_CORES = 8
QW = 512               # q-chunk width
KW = 128               # k-block width
SX, SW = 32.0, 512.0   # host-side fp8 scales for x and W
SCALE = 1.0 / math.sqrt(HD)


def _body(nc, tc, cfg, t):
    TOK = cfg["TOK"]
    DM = cfg["DM"]
    DC = DM // 128
    NP_ = DC // 2          # dc pairs
    NTC = TOK // QW        # chunks
    KB = TOK // KW         # k blocks
    inv = 1.0 / (SX * SW)

    with ExitStack() as es:
        P = es.enter_context(tc.tile_pool(name="persist", bufs=1))
        cpk = P.tile([128, TOK], FP16, tag="cpk", name="cpk")
        spk = P.tile([128, TOK], FP16, tag="spk", name="spk")
        negI = P.tile([64, 2, 128], FP8, tag="negI", name="negI")
        umask = P.tile([64, 4, 2, QW], FP8, tag="umask", name="umask")
        ones1 = P.tile([128, 1], FP16, tag="ones1", name="ones1")
        rotq = P.tile([128, NQ, TOK], FP16, tag="rotq", name="rotq")
        rotk = P.tile([128, NKV, TOK], FP16, tag="rotk", name="rotk")
        v_sb = P.tile([128, KB, NKV * HD], FP16, tag="v_sb", name="v_sb")

        work = es.enter_context(tc.tile_pool(name="work", bufs=1))
        pp = es.enter_context(tc.tile_pool(name="pp", bufs=1, space="PSUM"))

        # ---- constant loads (cpk first: warmup + rope) ----
        nc.sync.dma_start(out=cpk[:], in_=t["cpk"].ap()[:])
        nc.scalar.dma_start(out=spk[:], in_=t["spk"].ap()[:])
        nc.scalar.dma_start(out=negI[:], in_=t["negI"].ap()[:])
        nc.scalar.dma_start(out=umask[:], in_=t["umask"].ap()[:])
        nc.scalar.dma_start(out=ones1[:], in_=t["ones1"].ap()[:])

        wts = ExitStack()
        WP = wts.enter_context(tc.tile_pool(name="wts", bufs=1))
        wqh = WP.tile([128, DC, NQ * HD], FP8, tag="wqh", name="wqh")
        wql = WP.tile([128, DC, NQ * HD], FP8, tag="wql", name="wql")
        wkvh = WP.tile([128, DC, 2 * NKV * HD], FP8, tag="wkvh", name="wkvh")
        wkvl = WP.tile([128, DC, 2 * NKV * HD], FP8, tag="wkvl", name="wkvl")

        def load_chunk(c):
            xt = WP.tile([128, DC, 2 * QW], FP8, tag="xth", bufs=2, name="xt")
            ts = c * 2 * QW
            for dc in range(DC):
                eng = (nc.sync, nc.scalar, nc.gpsimd)[dc % 3]
                eng.dma_start(
                    out=xt[:, dc, :],
                    in_=t["x8"].ap()[dc * 128:(dc + 1) * 128,
                                     ts:ts + 2 * QW])
            return xt

        # PE p-state warmup: matmuls on a memset tile while loads run
        wum = pp.tile([128, 2 * QW], F32, tag="swath", bufs=2, name="wum")
        wsrc = work.tile([128, 640], FP16, tag="wsrc", bufs=1, name="wsrc")
        nc.vector.memset(wsrc[:], 0.5)
        for i in range(40):
            nc.tensor.matmul(wum[:, 0:QW], wsrc[:, 0:128], wsrc[:, 128:640],
                             start=True, stop=True, skip_group_check=True)

        xt0 = WP.tile([128, DC, 2 * QW], FP8, tag="xth", bufs=2, name="xt0")
        for dc in range(DC):
            # spread chunk-0/wkv loads across both HWDGE queues + SWDGE
            e1 = (nc.sync, nc.scalar, nc.gpsimd)[dc % 3]
            e2 = (nc.scalar, nc.gpsimd, nc.sync)[dc % 3]
            e4 = (nc.gpsimd, nc.sync, nc.scalar)[dc % 3]
            e1.dma_start(
                out=xt0[:, dc, :],
                in_=t["x8"].ap()[dc * 128:(dc + 1) * 128, 0:2 * QW])
            e2.dma_start(out=wkvh[:, dc, :],
                         in_=t["wkvh"].ap()[dc * 128:(dc + 1) * 128, :])
            e4.dma_start(
                out=wkvl[:, dc, :],
                in_=t["wkvl"].ap()[dc * 128:(dc + 1) * 128, :])
        for dc in range(DC):
            nc.sync.dma_start(out=wqh[:, dc, :],
                              in_=t["wqh"].ap()[dc * 128:(dc + 1) * 128, :])
            nc.gpsimd.dma_start(out=wql[:, dc, :],
                                in_=t["wql"].ap()[dc * 128:(dc + 1) * 128, :])

        def proj3(ps, wh, wl, xt, coff, cw, xoff=None, xw=None):
            """3-term fp8 residual projection over all dc pairs.

            xt holds [hi | lo] halves packed along the free dim.
            """
            plan = []
            for p in range(NP_):
                dcs = slice(2 * p, 2 * p + 2)
                if xoff is None:
                    plan.append((0, wh[:, dcs, coff:coff + cw],
                                 xt[:, dcs, 0:QW]))
                    plan.append((1, wl[:, dcs, coff:coff + cw],
                                 xt[:, dcs, 0:QW]))
                    plan.append((1, wh[:, dcs, coff:coff + cw],
                                 xt[:, dcs, QW:2 * QW]))
                else:
                    plan.append((0, xt[:, dcs, xoff:xoff + xw],
                                 wh[:, dcs, coff:coff + cw]))
                    plan.append((1, xt[:, dcs, QW + xoff:QW + xoff + xw],
                                 wh[:, dcs, coff:coff + cw]))
                    plan.append((1, xt[:, dcs, xoff:xoff + xw],
                                 wl[:, dcs, coff:coff + cw]))
            plan.sort(key=lambda it: it[0])  # all hi*hi terms first
            for i, (_, a, b_) in enumerate(plan):
                nc.tensor.matmul(ps, a, b_, start=(i == 0),
                                 stop=(i == len(plan) - 1), perf_mode=DR,
                                 skip_group_check=True)

        NH = NQ + NKV  # heads per chunk-batch of rope work (12)
        NHB = NH // 6  # rope dup processed in 6 head-batches of 2

        _pctr = [0]

        def proj_psum():
            _pctr[0] += 1
            tag = "popair" if _pctr[0] % 2 == 0 else "swath"
            tl = pp.tile([128, 2 * QW], F32, tag=tag, bufs=2, name="ps")
            return tl[:, 0:QW]

        def proj_head(wh, wl, coff, rawAll, hh, xt):
            """Project one q/k head; evacuate into rawAll[:, hh, :]."""
            ps = proj_psum()
            proj3(ps, wh, wl, xt, coff, HD)
            nc.scalar.activation(rawAll[:, hh, :], ps,
                                 mybir.ActivationFunctionType.Copy, scale=inv)

        def rope_batch(rawAll, qeAll, qoAll, dests, ts):
            """Duplicate even/odd halves for a head-batch, then rotate."""
            nc.gpsimd.dma_start(out=qeAll[0:64, :, :], in_=rawAll[0:64, :, :])
            nc.sync.dma_start(out=qeAll[64:128, :, :],
                               in_=rawAll[0:64, :, :])
            nc.gpsimd.dma_start(out=qoAll[0:64, :, :],
                                in_=rawAll[64:128, :, :])
            nc.sync.dma_start(out=qoAll[64:128, :, :],
                               in_=rawAll[64:128, :, :])
            for i, dest in enumerate(dests):
                t1 = work.tile([128, QW], FP16, tag="t1", bufs=1, name="t1")
                t2 = work.tile([128, QW], FP16, tag="t2", bufs=1, name="t2")
                nc.vector.tensor_mul(t1[:], qeAll[:, i, :], cpk[:, ts:ts + QW])
                nc.vector.tensor_mul(t2[:], qoAll[:, i, :], spk[:, ts:ts + QW])
                nc.vector.tensor_add(dest, t1[:], t2[:])

        # ---- wave A: chunks 0..NTC-2; last chunk interleaves into
        #      the ACT-bound early attention as PE filler ----
        def chunk_units(c, xt):
            """Generate per-chunk projection work as callable units."""
            ts = c * QW
            units = []
            for tb in range(QW // KW):
                def vproj(tb=tb):
                    psv = proj_psum()
                    proj3(psv, wkvh, wkvl, xt, NKV * HD, NKV * HD,
                          xoff=tb * KW, xw=KW)
                    nc.vector.tensor_scalar_mul(
                        v_sb[:, c * (QW // KW) + tb, :], psv, inv)
                units.append(vproj)
            heads = ([("kv", kv) for kv in range(NKV)]
                     + [("q", h) for h in range(NQ)])
            for hb in range(6):
                def ropeu(hb=hb):
                    batch = heads[hb * NHB:(hb + 1) * NHB]
                    rawAll = work.tile([128, NHB, QW], FP16, tag="rawAll",
                                       bufs=2, name="rawAll")
                    qeAll = work.tile([128, NHB, QW], FP16, tag="qeAll",
                                      bufs=2, name="qeAll")
                    qoAll = work.tile([128, NHB, QW], FP16, tag="qoAll",
                                      bufs=2, name="qoAll")
                    dests = []
                    for i, (kind, idx) in enumerate(batch):
                        if kind == "kv":
                            proj_head(wkvh, wkvl, idx * HD, rawAll, i, xt)
                            dests.append(rotk[:, idx, ts:ts + QW])
                        else:
                            proj_head(wqh, wql, idx * HD, rawAll, i, xt)
                            dests.append(rotq[:, idx, ts:ts + QW])
                    rope_batch(rawAll, qeAll, qoAll, dests, ts)
                units.append(ropeu)
            return units

        for c in range(NTC - 1):
            xtc = xt0 if c == 0 else load_chunk(c)
            for u in chunk_units(c, xtc):
                u()
        xt3 = load_chunk(NTC - 1)
        c3_units = chunk_units(NTC - 1, xt3)


        # -------- attention by head-pair, j outer; outproj interleaved ----
        pend = []

        def _finish(pr, j, poP, eaP):
            qs = j * QW
            pdP = work.tile([128, 2 * QW], FP16, tag="pdP", bufs=1,
                            name="pdP")
            nc.gpsimd.partition_all_reduce(pdP[:], eaP[:], 128,
                                           bass_isa.ReduceOp.add)
            bc = work.tile([128, 2 * QW], FP16, tag="bc", bufs=1, name="bc")
            with nc.allow_low_precision(reason="softmax rec fp16"):
                nc.vector.reciprocal(bc[:], pdP[:])
            for i in range(2):
                h = 2 * pr + i
                po = poP[:, i * QW:(i + 1) * QW]
                # attnT aliases rotq: pair (h, chunk j) slice is dead here
                nc.vector.tensor_mul(rotq[:, h, qs:qs + QW], po,
                                     bc[:, i * QW:(i + 1) * QW])

        def flush_pend(depth=1):
            while len(pend) > depth:
                _flush_one()

        def _flush_one():
            e, pr, j, blk, poP, eaP = pend.pop(0)
            kh = pr
            nblk = 4 * j + 4
            st, sp = (blk == 0), (blk == nblk - 1)
            for i in range(2):
                nc.tensor.matmul(poP[:, i * QW:(i + 1) * QW],
                                 v_sb[:, blk, kh * HD:(kh + 1) * HD],
                                 e[:, i * QW:(i + 1) * QW], start=st, stop=sp,
                                 skip_group_check=True)
            if st:
                nc.vector.tensor_copy(eaP[:], e[:])
            else:
                nc.vector.tensor_add(eaP[:], eaP[:], e[:])
            if sp:
                _finish(pr, j, poP, eaP)

        def attn_pair(pr, j):
            kh = pr
            qs = j * QW
            poP = eaP = None
            for blk in range(4 * j + 4):
                swt = pp.tile([128, 2 * QW], F32, tag="swath", bufs=2,
                              name="swt")
                diag = blk >= 4 * j
                for i in range(2):
                    nc.tensor.matmul(
                        swt[:, i * QW:(i + 1) * QW],
                        rotk[:, kh, blk * 128:(blk + 1) * 128],
                        rotq[:, 2 * pr + i, qs:qs + QW],
                        start=True, stop=not diag, skip_group_check=True)
                if diag:
                    di = blk - 4 * j
                    w = (di + 1) * 128
                    for i in range(2):
                        nc.tensor.matmul(
                            swt[:, i * QW:i * QW + w], negI[:],
                            umask[:, di, :, 0:w], start=False, stop=(i == 1),
                            perf_mode=DR, skip_group_check=True)
                e = work.tile([128, 2 * QW], FP16, tag="e", bufs=3, name="e")
                nc.scalar.activation(e[:], swt[:],
                                     mybir.ActivationFunctionType.Exp,
                                     scale=SCALE)
                flush_pend(depth=2)
                if blk == 0:
                    poP = pp.tile([128, 2 * QW], F32, tag="popair", bufs=2,
                                  name="poP")
                    eaP = work.tile([128, 2 * QW], FP16, tag="ea", bufs=2,
                                    name="eaP")
                pend.append((e, pr, j, blk, poP, eaP))

        def outproj_units(units):
            for tb, oc in units:
                ots = pp.tile([128, 2 * QW], F32, tag="swath", bufs=2,
                              name="ots")
                ot = ots[:, 0:QW]
                tsl = slice(tb * KW, (tb + 1) * KW)
                osl = slice(oc * QW, (oc + 1) * QW)
                for h in range(NQ):
                    nc.tensor.matmul(ot, rotq[:, h, tsl], wo16[:, h, osl],
                                     start=(h == 0), stop=(h == NQ - 1),
                                     skip_group_check=True)
                osb = work.tile([128, QW], F32, tag="osb", bufs=3,
                                name="osb")
                if (tb + oc) % 2 == 0:
                    nc.scalar.copy(osb[:], ot)
                    dma_eng = nc.sync
                else:
                    nc.vector.tensor_copy(osb[:], ot)
                    dma_eng = nc.scalar
                dma_eng.dma_start(
                    out=t["out"].ap()[tb * KW:(tb + 1) * KW,
                                      oc * QW:(oc + 1) * QW],
                    in_=osb[:])

        NOC = DM // QW

        def oust(j):
            return [(tb, oc) for tb in range(j * 4, (j + 1) * 4)
                    for oc in range(NOC)]

        # j0/j1: chunk-3 projection units as PE filler (2 per pair)
        for j in (0, 1):
            for pr in range(NKV):
                attn_pair(pr, j)
                for _ in range(2):
                    if c3_units:
                        c3_units.pop(0)()
        while c3_units:
            c3_units.pop(0)()
        # weights done; free their SBUF, then load Wo (fp16)
        wts.close()
        late = es.enter_context(tc.tile_pool(name="late", bufs=1))
        wo16 = late.tile([128, NQ, DM], FP16, tag="wo16", name="wo16")
        for h in range(NQ):
            eng = nc.sync if h % 2 == 0 else nc.scalar
            eng.dma_start(out=wo16[:, h, :],
                          in_=t["wo"].ap()[h * HD:(h + 1) * HD, :])
        # j2: outproj(0)+(1) as filler; j3: outproj(2)
        fill = oust(0) + oust(1)
        for pr in range(NKV):
            attn_pair(pr, 2)
            outproj_units(fill[pr * 8:(pr + 1) * 8])
        fill = oust(2)
        for pr in range(NKV):
            attn_pair(pr, 3)
            outproj_units(fill[pr * 4:(pr + 1) * 4])
        flush_pend(depth=0)
        outproj_units(oust(3))


def build(TOK=S, DM=D):
    cfg = dict(TOK=TOK, DM=DM)
    nc = bacc.Bacc("TRN2", target_bir_lowering=False, debug=False)
    t = {}
    t["x8"] = nc.dram_tensor("x8", [DM, 2 * TOK], FP8, kind="ExternalInput")
    t["wqh"] = nc.dram_tensor("wqh", [DM, NQ * HD], FP8, kind="ExternalInput")
    t["wql"] = nc.dram_tensor("wql", [DM, NQ * HD], FP8, kind="ExternalInput")
    t["wkvh"] = nc.dram_tensor("wkvh", [DM, 2 * NKV * HD], FP8,
                               kind="ExternalInput")
    t["wkvl"] = nc.dram_tensor("wkvl", [DM, 2 * NKV * HD], FP8,
                               kind="ExternalInput")
    t["wo"] = nc.dram_tensor("wo", [NQ * HD, DM], FP16, kind="ExternalInput")
    t["cpk"] = nc.dram_tensor("cpk", [128, TOK], FP16, kind="ExternalInput")
    t["spk"] = nc.dram_tensor("spk", [128, TOK], FP16, kind="ExternalInput")
    t["negI"] = nc.dram_tensor("negI", [64, 2 * 128], FP8,
                               kind="ExternalInput")
    t["umask"] = nc.dram_tensor("umask", [64, 4 * 2 * QW], FP8,
                                kind="ExternalInput")
    t["ones1"] = nc.dram_tensor("ones1", [128, 1], FP16,
                                kind="ExternalInput")
    t["out"] = nc.dram_tensor("out", [TOK, DM], F32, kind="ExternalOutput")
    with tile.TileContext(nc) as tc:
        _body(nc, tc, cfg, t)
    nc.compile()
    return nc


# ---------------- host-side sharding ----------------

def _rope_perm():
    return np.concatenate([np.arange(0, 128, 2), np.arange(1, 128, 2)])


def _res(v):
    hi = v.astype(F8NP)
    lo = (v - hi.astype(np.float32)).astype(F8NP)
    return hi, lo


def _consts():
    negI = np.zeros((64, 2, 128), np.float32)
    for sl in range(2):
        for r in range(64):
            negI[r, sl, r + 64 * sl] = -15.0
    kk = np.arange(128)[:, None]
    qq = np.arange(QW)[None, :]
    umask = np.zeros((64, 4, 2, QW), np.float32)
    for di in range(4):
        u = ((di * 128 + kk) > qq) * 15.0
        umask[:, di, 0, :] = u[0:64]
        umask[:, di, 1, :] = u[64:128]
    return (negI.reshape(64, 256).astype(F8NP),
            umask.reshape(64, 4 * 2 * QW).astype(F8NP))


def shard_inputs(x, freqs_cos, freqs_sin, Wq, Wk, Wv, Wo):
    perm = _rope_perm()
    negI, umask = _consts()
    cpk = np.concatenate([freqs_cos.T, freqs_sin.T], 0).astype(np.float16)
    spk = np.concatenate([-freqs_sin.T, freqs_cos.T], 0).astype(np.float16)
    ones1 = np.ones((128, 1), np.float16)

    in_maps = []
    for b in range(B):
        xt = np.ascontiguousarray(np.asarray(x)[b].T).astype(np.float32) * SX
        xhi, xlo = _res(xt)
        x8 = np.empty((D, 2 * S), F8NP)
        for c in range(S // QW):
            x8[:, c * 2 * QW:c * 2 * QW + QW] = xhi[:, c * QW:(c + 1) * QW]
            x8[:, c * 2 * QW + QW:(c + 1) * 2 * QW] = (
                xlo[:, c * QW:(c + 1) * QW])
        for g in range(NG):
            qh = slice(g * NQ * HD, (g + 1) * NQ * HD)
            kvh = slice(g * NKV * HD, (g + 1) * NKV * HD)
            wq_g = (Wq[:, qh].reshape(D, NQ, HD)[:, :, perm]
                    .reshape(D, NQ * HD).astype(np.float32) * SW)
            wk_g = (Wk[:, kvh].reshape(D, NKV, HD)[:, :, perm]
                    .reshape(D, NKV * HD).astype(np.float32) * SW)
            wkv = np.concatenate([wk_g, Wv[:, kvh].astype(np.float32) * SW],
                                 axis=1)
            wqh_, wql_ = _res(np.ascontiguousarray(wq_g))
            wkvh_, wkvl_ = _res(np.ascontiguousarray(wkv))
            in_maps.append(dict(
                x8=x8, wqh=wqh_, wql=wql_, wkvh=wkvh_, wkvl=wkvl_,
                wo=np.ascontiguousarray(Wo[qh, :]).astype(np.float16),
                cpk=cpk, spk=spk, negI=negI, umask=umask, ones1=ones1,
            ))
    return in_maps


_NC_CACHE = {}


def kernel(x, freqs_cos, freqs_sin, Wq, Wk, Wv, Wo):
    """Full-problem entry point: full inputs in, full [B,S,D] fp32 out."""
    if "nc" not in _NC_CACHE:
        _NC_CACHE["nc"] = build()
    nc = _NC_CACHE["nc"]
    in_maps = shard_inputs(
        np.asarray(x), np.asarray(freqs_cos), np.asarray(freqs_sin),
        np.asarray(Wq), np.asarray(Wk), np.asarray(Wv), np.asarray(Wo),
    )
    res = run_bass_kernel_spmd(nc, in_maps, core_ids=list(range(N_CORES)))
    out = np.zeros((B, S, D), np.float32)
    for b in range(B):
        out[b] = res.results[b * NG]["out"] + res.results[b * NG + 1]["out"]
    return out

